# revision 3
# baseline (speedup 1.0000x reference)
"""Trainium2 Bass kernel for GPLinear — Pauli rep + Karatsuba complex mult.

v4: same math as v3 (M2(C) factorization + 3-mult complex product:
m1 = aRe*bRe, m2 = aIm*bIm, m3 = (aRe+aIm)(bRe+bIm); recombination on
host), restructured for overlap:

  - MMs ordered by (v,r) output group: 6 groups x 8 MMs (t,s), each
    group accumulating into its own PSUM bank tile.  Evac (ACT) of a
    group runs while the next group's MMs stream -> no end-of-body
    evac bubble, no PSUM bank cycling.
  - One contiguous DMA per tensor per body: xh (1 MB) on the SP HWDGE
    ring, wh (2 MB) on the ACT HWDGE ring; stores batched 2 groups at
    a time on the GPSIMD SWDGE ring.
  - Karatsuba Sum coords formed by single strided DVE adds into
    separate Sum tiles (DVE has ~6.7us of work vs ~13us PE per body).

Sharding (8 cores): 4-way batch x 2-way out_features.
"""

import numpy as np
import ml_dtypes

import concourse.bass as bass
import concourse.mybir as mybir
import concourse.tile as tile
from concourse import bacc
from concourse.bass_utils import run_bass_kernel_spmd

F32 = mybir.dt.float32
BF16 = mybir.dt.bfloat16
BF16_NP = ml_dtypes.bfloat16

BATCH, IN_F, OUT_F, K8 = 512, 512, 512, 8
R_B, R_O = 4, 2
N_CORES = R_B * R_O
B_LOC = BATCH // R_B
OC = OUT_F // R_O
PT = IN_F // 128

LAST_RESULTS = None


def _cayley_table() -> np.ndarray:
    G = np.zeros((8, 8, 8), dtype=np.float32)
    for a in range(8):
        for b in range(8):
            swaps, t = 0, a >> 1
            while t:
                swaps += bin(t & b).count("1")
                t >>= 1
            G[a, b, a ^ b] = -1.0 if (swaps & 1) else 1.0
    return G


def _check_G(G):
    assert np.array_equal(np.asarray(G, dtype=np.float32), _cayley_table()), \
        "G is not the Cl(3,0) Cayley table this kernel hardcodes"


def _T8():
    T = np.zeros((8, 8), dtype=np.float32)
    rows = {
        (0, 0, 0): [(0, 1), (4, 1)],
        (0, 0, 1): [(3, 1), (7, 1)],
        (0, 1, 0): [(1, 1), (5, -1)],
        (0, 1, 1): [(6, 1), (2, -1)],
        (1, 0, 0): [(1, 1), (5, 1)],
        (1, 0, 1): [(2, 1), (6, 1)],
        (1, 1, 0): [(0, 1), (4, -1)],
        (1, 1, 1): [(7, 1), (3, -1)],
    }
    for (r, s, u), terms in rows.items():
        for i, coef in terms:
            T[r * 4 + s * 2 + u, i] = coef
    return T


def _S8():
    S = np.zeros((8, 8), dtype=np.float32)
    outrows = {
        0: [((0, 0, 0), 1), ((1, 1, 0), 1)],
        4: [((0, 0, 0), 1), ((1, 1, 0), -1)],
        3: [((0, 0, 1), 1), ((1, 1, 1), -1)],
        7: [((0, 0, 1), 1), ((1, 1, 1), 1)],
        1: [((0, 1, 0), 1), ((1, 0, 0), 1)],
        5: [((1, 0, 0), 1), ((0, 1, 0), -1)],
        2: [((1, 0, 1), 1), ((0, 1, 1), -1)],
        6: [((0, 1, 1), 1), ((1, 0, 1), 1)],
    }
    for k, terms in outrows.items():
        for (r, c, u), coef in terms:
            S[k, r * 4 + c * 2 + u] = coef
    return S


def build_kernel(G, loop_n=None, variant="full"):
    _check_G(G)
    nc = bacc.Bacc("TRN2", target_bir_lowering=False, debug=False)

    XW = PT * 8 * B_LOC    # xh dma cols: (t, (r,s), ReIm, b)
    WW = PT * 8 * OC       # wh dma cols: (t, (s,c), ReIm, o)
    XSW = PT * 4 * B_LOC   # xsum cols:   (t, (r,s), b)
    WSW = PT * 4 * OC      # wsum cols:   (t, (s,c), o)

    xh_d = nc.dram_tensor("xh", [128, XW], BF16, kind="ExternalInput")
    wh_d = nc.dram_tensor("wh", [128, WW], BF16, kind="ExternalInput")
    o_d = nc.dram_tensor("out", [B_LOC, 12 * OC], BF16, kind="ExternalOutput")

    import contextlib

    NB = 2  # SBUF double-buffer sets; 6 PSUM bank tiles are shared

    with tile.TileContext(nc) as tc:
        with (
            tc.tile_pool(name="sb", bufs=1) as sb,
            tc.tile_pool(name="ps", bufs=1, space="PSUM") as ps,
        ):
            xh_t = [sb.tile([128, XW], BF16, tag=f"xh{j}", name=f"xh{j}")
                    for j in range(NB)]
            wh_t = [sb.tile([128, WW], BF16, tag=f"wh{j}", name=f"wh{j}")
                    for j in range(NB)]
            xs_t = [sb.tile([128, XSW], BF16, tag=f"xs{j}", name=f"xs{j}")
                    for j in range(NB)]
            ws_t = [sb.tile([128, WSW], BF16, tag=f"ws{j}", name=f"ws{j}")
                    for j in range(NB)]
            out_sb = [sb.tile([128, 12 * OC], BF16, tag=f"out{j}",
                              name=f"out{j}") for j in range(NB)]
            pst = [ps.tile([128, 512], F32, tag=f"ps{g}", name=f"ps{g}")
                   for g in range(6)]

            def do_dma(j):
                nc.sync.dma_start(xh_t[j][:], xh_d.ap())
                nc.scalar.dma_start(wh_t[j][:], wh_d.ap())

            def do_sums(j):
                x_, w_ = xh_t[j], wh_t[j]
                xs_, ws_ = xs_t[j], ws_t[j]
                px, pw = x_[:].ap[0][0], w_[:].ap[0][0]
                B = B_LOC
                x_re = bass.AP(tensor=x_.tensor, offset=0,
                               ap=[[px, 128], [8 * B, PT], [2 * B, 4], [1, B]])
                x_im = bass.AP(tensor=x_.tensor, offset=B,
                               ap=[[px, 128], [8 * B, PT], [2 * B, 4], [1, B]])
                nc.vector.tensor_tensor(out=xs_[:], in0=x_re, in1=x_im,
                                        op=mybir.AluOpType.add)
                w_re = bass.AP(tensor=w_.tensor, offset=0,
                               ap=[[pw, 128], [8 * OC, PT], [2 * OC, 4],
                                   [1, OC]])
                w_im = bass.AP(tensor=w_.tensor, offset=OC,
                               ap=[[pw, 128], [8 * OC, PT], [2 * OC, 4],
                                   [1, OC]])
                nc.vector.tensor_tensor(out=ws_[:], in0=w_re, in1=w_im,
                                        op=mybir.AluOpType.add)

            # group g = v*2 + r; 8 MMs (t, s) accumulate into bank tile g
            def mm_group(j, g):
                v, r = divmod(g, 2)
                x_, w_ = xh_t[j], wh_t[j]
                xs_, ws_ = xs_t[j], ws_t[j]
                px, pw = x_[:].ap[0][0], w_[:].ap[0][0]
                pxs, pws = xs_[:].ap[0][0], ws_[:].ap[0][0]
                B = B_LOC
                for t in range(PT):
                    for s in range(2):
                        first = (t == 0 and s == 0)
                        last = (t == PT - 1 and s == 1)
                        if v < 2:
                            lhsT = bass.AP(
                                tensor=x_.tensor,
                                offset=t * 8 * B + (r * 2 + s) * 2 * B + v * B,
                                ap=[[px, 128], [1, B]])
                            rhs = bass.AP(
                                tensor=w_.tensor,
                                offset=t * 8 * OC + s * 4 * OC + v * OC,
                                ap=[[pw, 128], [2 * OC, 2], [1, OC]])
                        else:
                            lhsT = bass.AP(
                                tensor=xs_.tensor,
                                offset=t * 4 * B + (r * 2 + s) * B,
                                ap=[[pxs, 128], [1, B]])
                            rhs = bass.AP(
                                tensor=ws_.tensor,
                                offset=t * 4 * OC + s * 2 * OC,
                                ap=[[pws, 128], [1, 2 * OC]])
                        nc.tensor.matmul(pst[g][:], lhsT, rhs,
                                         start=first, stop=last)

            def do_evac(j, g):
                nc.scalar.copy(out_sb[j][:, g * 512:(g + 1) * 512], pst[g][:])

            def do_store(j, g01):
                lo = g01 * 1024
                nc.gpsimd.dma_start(o_d.ap()[:, lo:lo + 1024],
                                    out_sb[j][:, lo:lo + 1024])

            def body(j, store=True):
                do_dma(j)
                do_sums(j)
                for g in range(6):
                    mm_group(j, g)
                    do_evac(j, g)
                    if store and g % 2 == 1:
                        do_store(j, g // 2)

            if loop_n:
                assert loop_n % NB == 0, f"loop_n must be a multiple of {NB}"
            loop = (tc.For_i(0, loop_n // NB, 1) if loop_n
                    else contextlib.nullcontext())
            if variant == "full":
                if not loop_n:
                    body(0)
                else:
                    with loop:
                        for j in range(NB):
                            body(j)
            elif variant == "mm":
                for j in range(NB):
                    do_dma(j)
                with loop:
                    for j in range(NB):
                        do_sums(j)
                        for g in range(6):
                            mm_group(j, g)
                            do_evac(j, g)
            elif variant == "dma":
                with loop:
                    for j in range(NB):
                        do_dma(j)
            else:
                raise ValueError(variant)

    nc.compile()
    return nc


def _host_transform(x, W, b=None):
    x = np.asarray(x, dtype=np.float32)
    W = np.asarray(W, dtype=np.float32)
    T8 = _T8()

    xh8 = np.einsum("bpi,ai->bpa", x, T8).astype(BF16_NP)   # [B,P,8] (r,s,u)
    wh8 = np.einsum("poj,aj->poa", W, 0.5 * T8).astype(BF16_NP)  # (s,c,u')

    in_maps = []
    for c in range(N_CORES):
        bc, oc = divmod(c, R_O)
        xh_c = xh8[bc * B_LOC:(bc + 1) * B_LOC]           # [128, 512, 8]
        xh_c = np.ascontiguousarray(
            xh_c.transpose(1, 2, 0)                        # [512, 8, 128]
                .reshape(PT, 128, 8, B_LOC)                # [t, p, (r,s,u), b]
                .transpose(1, 0, 2, 3)
                .reshape(128, PT * 8 * B_LOC))
        wh_c = wh8[:, oc * OC:(oc + 1) * OC, :]           # [512, 256, 8]
        wh_c = np.ascontiguousarray(
            wh_c.transpose(0, 2, 1)                        # [512, 8, 256]
                .reshape(PT, 128, 8, OC)                   # [t, p, (s,c,u'), o]
                .transpose(1, 0, 2, 3)
                .reshape(128, PT * 8 * OC))
        in_maps.append({"xh": xh_c, "wh": wh_c})
    return in_maps


def make_in_maps(x, W, b, G=None):
    return _host_transform(x, W, b)


_CACHE = {}


def kernel(x, W, b, G):
    global LAST_RESULTS
    _check_G(G)
    if "nc" not in _CACHE:
        _CACHE["nc"] = build_kernel(G)
    nc = _CACHE["nc"]

    in_maps = _host_transform(x, W)
    res = run_bass_kernel_spmd(nc, in_maps, core_ids=list(range(N_CORES)))
    LAST_RESULTS = res

    S8 = _S8()
    b = np.asarray(b, dtype=np.float32)
    out = np.empty((BATCH, OUT_F, K8), dtype=np.float32)
    for c in range(N_CORES):
        bc, oc = divmod(c, R_O)
        M = np.asarray(res.results[c]["out"]).astype(np.float32)
        M = M.reshape(B_LOC, 3, 2, 2, OC)                 # [b, v, r, c, o]
        O = np.empty((B_LOC, 2, 2, 2, OC), dtype=np.float32)  # [b,r,c,u'',o]
        O[:, :, :, 0] = M[:, 0] - M[:, 1]                 # m1 - m2
        O[:, :, :, 1] = M[:, 2] - M[:, 0] - M[:, 1]       # m3 - m1 - m2
        O = O.reshape(B_LOC, 8, OC)                       # gamma = (r,c,u'')
        o_c = np.einsum("kg,bgo->bok", S8, O) + b[oc * OC:(oc + 1) * OC]
        out[bc * B_LOC:(bc + 1) * B_LOC, oc * OC:(oc + 1) * OC, :] = o_c
    return out
